# revision 3
# baseline (speedup 1.0000x reference)
"""Trainium2 Bass kernel for nn_CZT_prop, v2: host-precomputed trig planes,
hybrid 4-mult/Karatsuba complex matmuls, multi-engine elementwise.

Math (per wavelength, per column-half core):
    out = F0 . (E^T (U0^T E_half)) * s,   U0 = field . F   (. = elementwise)
with E the sigma-permuted chirp matrix carrying b/gamma phases (same
construction as baseline), F/F0 the quadrant-tiled RS kernels.

All wavelength-dependent trig (E planes, F/F0 quads) is evaluated on host in
f64 and shipped as f16 (same category as the baseline's btq/aaq/eslope
tables). Device does only field-dependent work:
  U0   = field . F                      (DVE + GpSimd tensor_tensor)
  X1   = U0^T E[:, :512]                (PE; mt 0-3 classic 4-mult with -Ei
                                         plane, mt 4-7 Karatsuba 3-mult)
  X2   = E^T X1                         (PE; Karatsuba 3-mult)
  Y    = F0 . X2                        (DVE + GpSimd)
Scales folded into F on host so PSUM values drain to f16 with no scalar
multiplies on device. PSUM rule: each combine reads at most ONE PSUM operand
(hardware constraint), so drains are ACT copy + stt/tt combines.
"""
import math
import numpy as np

f32 = np.float32
f16 = np.float16
f64 = np.float64

# ---- static geometry (mirrors the problem spec) ----
H = 1024
O_H = 1024
N_WL = 4
DX = 100e-6
ODX = 10e-6
ODY = 10e-6
Z = 0.05
TWO_PI = 2.0 * np.pi
M = 1024
P = 128
NB = 8          # partition blocks per plane
HN = 512        # half width

X_IN = np.linspace(-H * DX / 2, H * DX / 2, H).astype(f64)
X_OUT = np.linspace(-O_H * ODX / 2, O_H * ODX / 2, O_H).astype(f64)
SIGMA = np.concatenate([np.arange(512), np.arange(1023, 511, -1)])  # sigma(s)
C0 = Z / TWO_PI
X2TARGET = 16384.0


def _pow2_below(x):
    return 2.0 ** math.floor(math.log2(x))


def host_prepare(field_real, field_imag, wavelengths):
    """Build per-core input maps + output assembly metadata. All f64 host math."""
    wls = np.asarray(wavelengths, f64)
    maxfield = float(max(np.abs(field_real).max(), np.abs(field_imag).max(), 1e-30))
    C_FIELD = _pow2_below(0.25 / maxfield)

    # input/output quadrant geometry (shared)
    def quad_geom(xg):
        r2 = xg[:512, None] ** 2 + xg[None, :512] ** 2 + Z * Z
        r = np.sqrt(r2)
        return r, 1.0 / r2, 1.0 / (r2 * r)
    r_in, i2_in, i3_in = quad_geom(X_IN)
    r_out, i2_out, i3_out = quad_geom(X_OUT)

    perm_fields = {}
    for w in range(N_WL):
        fr = np.asarray(field_real[0, w], f64)[SIGMA][:, SIGMA]
        fi = np.asarray(field_imag[0, w], f64)[SIGMA][:, SIGMA]
        perm_fields[w] = (f16(fr * C_FIELD), f16(fi * C_FIELD))

    JCOL = np.arange(HN, dtype=f64)
    in_maps = []
    meta = []
    for core in range(8):
        w, bh = core // 2, core % 2
        wl = f64(wls[w])
        Dm = wl * Z / DX
        fx1 = X_OUT[0] + Dm / 2
        fx2 = X_OUT[-1] + Dm / 2
        D1 = fx1 + (M * Dm + fx2 - fx1) / (2 * M)
        D2 = fx2 + (M * Dm + fx2 - fx1) / (2 * M)
        alpha = TWO_PI * (D2 - D1) / (M * Dm)
        kwav = TWO_PI / wl
        gam1 = TWO_PI * (M - 1) * (D2 - D1) / (2 * Dm * M) - alpha
        gam0 = TWO_PI * (M - 1) * D1 / (2 * Dm) - alpha / 2
        beta = alpha - TWO_PI * D1 / Dm

        # ---- E plane [1024, 1024] (rows s, cols 512h + j) ----
        sg = SIGMA.astype(f64)
        er = np.empty((1024, 1024), f16)
        ei = np.empty((1024, 1024), f16)
        es = np.empty((1024, 1024), f16)
        for h in range(2):
            par = (h + bh) % 2
            sgn = 1.0 - 2.0 * par
            base = 1023.0 * par
            sl = sgn * (alpha * sg + gam1)
            of = (alpha * sg + gam1) * base + beta * sg + gam0
            ph = JCOL[None, :] * sl[:, None] + of[:, None]
            c = np.cos(ph)
            s = np.sin(ph)
            er[:, HN * h:HN * (h + 1)] = f16(c)
            ei[:, HN * h:HN * (h + 1)] = f16(s)
            es[:, HN * h:HN * (h + 1)] = f16(c + s)
        # zm fixup: exactly one of E[0,0] / E[0,512] is kept per half-parity
        zcol = HN * (1 - bh)
        er[0, zcol] = 0
        ei[0, zcol] = 0
        es[0, zcol] = 0
        nei_l = f16(-ei[:, 0:HN])

        # ---- dynamic scale fold ----
        bmax = kwav * C0 / (Z * Z)
        amax = C0 / (Z * Z) ** 1.5
        Fmax = math.sqrt(bmax * bmax + amax * amax)
        C_F = _pow2_below(X2TARGET / ((1024.0 * 1.42) ** 2
                                      * maxfield * C_FIELD * Fmax * 1.5))
        s_w = Z * ODX * ODY * wl
        s_eff = s_w / (C_FIELD * C_F)

        # ---- F quad (input plane), scaled by C_F; duplicated [q | q] ----
        def f_planes(r, i2, i3, scale):
            kr = kwav * r
            ckr = np.cos(kr)
            skr = np.sin(kr)
            fre = (C0 * scale) * (i3 * ckr + kwav * i2 * skr)
            fim = (C0 * scale) * (i3 * skr - kwav * i2 * ckr)
            return fre, fim
        fre_q, fim_q = f_planes(r_in, i2_in, i3_in, C_F)
        f0re_q, f0im_q = f_planes(r_out, i2_out, i3_out, s_eff)
        assert np.abs(f0re_q).max() < 30000 and np.abs(f0im_q).max() < 30000, \
            (np.abs(f0re_q).max(), np.abs(f0im_q).max())
        fqr = f16(np.tile(fre_q, (1, 4)))   # [512, 2048] = [Fq|Fq|Fq|Fq]
        fqi = f16(np.tile(fim_q, (1, 4)))
        f0r = f16(f0re_q)                   # [512, 512] quad
        f0i = f16(f0im_q)

        fr16, fi16 = perm_fields[w]
        eall = np.concatenate([er, ei, nei_l], axis=1)          # [1024, 2560]
        # field pair rows: block p = [fr_p | fr_p+4 | fi_p | fi_p+4]  [512, 4096]
        fieldall = np.concatenate(
            [np.concatenate([fr16[128 * p:128 * (p + 1)],
                             fr16[128 * (p + 4):128 * (p + 5)],
                             fi16[128 * p:128 * (p + 1)],
                             fi16[128 * (p + 4):128 * (p + 5)]], axis=1)
             for p in range(4)], axis=0)
        f0all = np.concatenate(
            [np.hstack([f0r[128 * q:128 * (q + 1)] for q in range(4)]),
             np.hstack([f0i[128 * q:128 * (q + 1)] for q in range(4)])],
            axis=1)                                             # [128, 4096]
        in_maps.append({
            "fieldall": fieldall, "eall": eall, "es": es,
            "fqr": fqr, "fqi": fqi, "f0all": f0all,
        })
        rmap = SIGMA[(np.arange(1024) + 512 * bh) % 1024]
        cmap = np.arange(512) if bh == 0 else 1023 - np.arange(512)
        meta.append((w, rmap, cmap))
    return in_maps, meta


def assemble(results, meta):
    out = np.zeros((1, N_WL, O_H, O_H), np.complex64)
    for core, (w, rmap, cmap) in enumerate(meta):
        yb = results[core]["y"] if "y" in results[core] else None
        if yb is not None:
            y = yb[:, 0:512].astype(f32) + 1j * yb[:, 512:1024].astype(f32)
        else:
            y = results[core]["yre"].astype(f32) + 1j * results[core]["yim"].astype(f32)
        out[0, w][np.ix_(cmap, rmap)] = y.T
    return out


# ---------------- golden (numpy) model of the device program ----------------

def golden_core(inp):
    fa = inp["fieldall"]
    fr = np.concatenate([np.concatenate([fa[128 * p:128 * (p + 1), 0:1024] for p in range(4)], axis=0),
                         np.concatenate([fa[128 * p:128 * (p + 1), 1024:2048] for p in range(4)], axis=0)], axis=0)
    fi = np.concatenate([np.concatenate([fa[128 * p:128 * (p + 1), 2048:3072] for p in range(4)], axis=0),
                         np.concatenate([fa[128 * p:128 * (p + 1), 3072:4096] for p in range(4)], axis=0)], axis=0)
    er = inp["eall"][:, 0:1024]; ei = inp["eall"][:, 1024:2048]
    nei = inp["eall"][:, 2048:2560]; es = inp["es"]
    fqr = inp["fqr"]; fqi = inp["fqi"]
    # U0 per block: products then combines, all f16-rounded
    U0r = np.empty((1024, 1024), f16)
    U0i = np.empty((1024, 1024), f16)
    U0s = np.empty((1024, 1024), f16)
    for kt in range(NB):
        q = kt % 4
        sl = slice(P * kt, P * (kt + 1))
        qsl = slice(P * q, P * (q + 1))
        Fr = fqr[qsl, 0:1024].astype(f32); Fi = fqi[qsl, 0:1024].astype(f32)
        a = fr[sl].astype(f32); b = fi[sl].astype(f32)
        p1 = f16(a * Fr); p2 = f16(b * Fi)
        p3 = f16(a * Fi); p4 = f16(b * Fr)
        U0r[sl] = f16(p1.astype(f32) - p2.astype(f32))
        U0i[sl] = f16(p3.astype(f32) + p4.astype(f32))
        U0s[sl] = f16(U0r[sl].astype(f32) + U0i[sl].astype(f32))

    def mm(A, B):
        return A.astype(f32) @ B.astype(f32)

    X1r = np.empty((1024, HN), f16)
    X1i = np.empty((1024, HN), f16)
    X1s = np.empty((1024, HN), f16)
    er_l = er[:, 0:HN]; ei_l = ei[:, 0:HN]; es_l = es[:, 0:HN]
    for mt in range(NB):
        msl = slice(P * mt, P * (mt + 1))
        if mt < 4:
            psR = mm(U0r[:, msl].T, er_l) + mm(U0i[:, msl].T, nei)
            psI = mm(U0r[:, msl].T, ei_l) + mm(U0i[:, msl].T, er_l)
            X1r[msl] = f16(psR)
            X1i[msl] = f16(psI)
            X1s[msl] = f16(X1r[msl].astype(f32) + X1i[msl].astype(f32))
        else:
            psA = mm(U0r[:, msl].T, er_l)
            psB = mm(U0i[:, msl].T, ei_l)
            psC = mm(U0s[:, msl].T, es_l)
            b_ = f16(psB)
            X1r[msl] = f16(psA - b_.astype(f32))              # stt: psA - b
            X1s[msl] = f16(psC - 2.0 * b_.astype(f32))        # stt: psC - 2b
            X1i[msl] = f16(X1s[msl].astype(f32) - X1r[msl].astype(f32))

    X2r = np.empty((1024, HN), f16)
    X2i = np.empty((1024, HN), f16)
    for mt in range(NB):
        msl = slice(P * mt, P * (mt + 1))
        psA2 = mm(er[:, msl].T, X1r)
        psB2 = mm(ei[:, msl].T, X1i)
        psC2 = mm(es[:, msl].T, X1s)
        a2 = f16(psA2)
        X2r[msl] = f16(a2.astype(f32) - psB2)                  # stt
        t = f16(psC2 - a2.astype(f32))                         # stt: m3 - m1
        X2i[msl] = f16(t.astype(f32) - psB2)                   # stt

    f0r = inp["f0all"][:, 0:2048]; f0i = inp["f0all"][:, 2048:4096]
    Yre = np.empty((1024, HN), f16)
    Yim = np.empty((1024, HN), f16)
    for m2 in range(NB):
        q = m2 % 4
        msl = slice(P * m2, P * (m2 + 1))
        F0r = f0r[:, HN * q:HN * (q + 1)].astype(f32)
        F0i = f0i[:, HN * q:HN * (q + 1)].astype(f32)
        x2r = X2r[msl].astype(f32)
        x2i = X2i[msl].astype(f32)
        t1 = f16(F0r * x2r); t2 = f16(F0i * x2i)
        t3 = f16(F0r * x2i); t4 = f16(F0i * x2r)
        Yre[msl] = f16(t1.astype(f32) - t2.astype(f32))
        Yim[msl] = f16(t3.astype(f32) + t4.astype(f32))
    return {"yre": Yre, "yim": Yim}


def golden(field_real, field_imag, wavelengths):
    in_maps, meta = host_prepare(field_real, field_imag, wavelengths)
    results = [golden_core(m) for m in in_maps]
    return assemble(results, meta)


# ---------------- bass program ----------------

_PROGRAM = None

KT_ORDER = [0, 4, 1, 5, 2, 6, 3, 7]   # pair production order: (0,4),(1,5),...


def build_program():
    import concourse.bass as bass
    import concourse.tile as tile
    import concourse.mybir as mybir
    from concourse import bacc

    dt = mybir.dt
    AF = mybir.ActivationFunctionType
    ALU = mybir.AluOpType

    nc = bacc.Bacc("TRN2", target_bir_lowering=False, debug=False, num_devices=8)

    fieldall_d = nc.dram_tensor("fieldall", [512, 4096], dt.float16, kind="ExternalInput").ap()
    eall_d = nc.dram_tensor("eall", [1024, 2560], dt.float16, kind="ExternalInput").ap()
    es_d = nc.dram_tensor("es", [1024, 1024], dt.float16, kind="ExternalInput").ap()
    fqr_d = nc.dram_tensor("fqr", [512, 2048], dt.float16, kind="ExternalInput").ap()
    fqi_d = nc.dram_tensor("fqi", [512, 2048], dt.float16, kind="ExternalInput").ap()
    f0all_d = nc.dram_tensor("f0all", [128, 4096], dt.float16, kind="ExternalInput").ap()
    y_d = nc.dram_tensor("y", [1024, 1024], dt.float16, kind="ExternalOutput").ap()

    with tile.TileContext(nc) as tc:
      with tc.tile_pool(name="persist", bufs=1) as pp, \
           tc.tile_pool(name="fld", bufs=1) as fld, \
           tc.tile_pool(name="u0t", bufs=1) as u0t, \
           tc.tile_pool(name="tmp1", bufs=1) as tp1, \
           tc.tile_pool(name="tmp", bufs=2) as tp, \
           tc.tile_pool(name="psum", bufs=1, space="PSUM") as psp:

        # ---- persistent tiles ----
        E = [pp.tile([P, 2560], dt.float16, tag=f"E{t}", name=f"E{t}") for t in range(NB)]
        fq_r = [pp.tile([P, 2048], dt.float16, tag=f"fqr{q}", name=f"fqr{q}") for q in range(4)]
        fq_i = [pp.tile([P, 2048], dt.float16, tag=f"fqi{q}", name=f"fqi{q}") for q in range(4)]
        f0 = pp.tile([P, 4096], dt.float16, tag="f0", name="f0")
        U0r = [pp.tile([P, 2048], dt.float16, tag=f"u0r{i}", name=f"u0r{i}") for i in range(4)]
        U0i = [pp.tile([P, 2048], dt.float16, tag=f"u0i{i}", name=f"u0i{i}") for i in range(4)]
        U0s = [pp.tile([P, 2048], dt.float16, tag=f"u0s{i}", name=f"u0s{i}") for i in range(4)]
        X1r = [pp.tile([P, HN], dt.float16, tag=f"x1r{t}", name=f"x1r{t}") for t in range(NB)]
        X1i = [pp.tile([P, HN], dt.float16, tag=f"x1i{t}", name=f"x1i{t}") for t in range(NB)]
        X1s = [pp.tile([P, HN], dt.float16, tag=f"x1s{t}", name=f"x1s{t}") for t in range(NB)]

        def er_ap(kt, lo, hi):
            return E[kt][:, lo:hi]

        def ei_ap(kt, lo, hi):
            return E[kt][:, 1024 + lo:1024 + hi]

        def nei_ap(kt):
            return E[kt][:, 2048:2560]

        def u0slice(plane, kt, mt):
            base = 1024 * (kt // 4)
            return plane[kt % 4][:, base + P * mt: base + P * (mt + 1)]

        es_pair = None

        def es_ap(kt, lo, hi):
            base = 1024 * (kt // 4)
            return es_pair[kt % 4][:, base + lo: base + hi]

        # ---- input DMAs: per-pair interleave on gpsimd queue ----
        fb_t = []
        for i in range(4):
            nc.gpsimd.dma_start(fq_r[i][:], fqr_d[P * i:P * (i + 1), :])
            fbt = fld.tile([P, 4096], dt.float16, tag=f"fb{i}", name=f"fb{i}")
            nc.gpsimd.dma_start(fbt[:], fieldall_d[P * i:P * (i + 1), :])
            nc.gpsimd.dma_start(fq_i[i][:], fqi_d[P * i:P * (i + 1), :])
            fb_t.append(fbt)
            nc.gpsimd.dma_start(E[i][:], eall_d[P * i:P * (i + 1), :])
            nc.gpsimd.dma_start(E[i + 4][:], eall_d[P * (i + 4):P * (i + 5), :])

        # ---- U0 production on DVE, pair-blocks [128,2048] ----
        for i in range(4):
            fbt = fb_t[i]
            fr_ap = fbt[:, 0:2048]
            fi_ap = fbt[:, 2048:4096]
            p1 = u0t.tile([P, 2048], dt.float16, tag="pA", name=f"p1_{i}")
            p2 = u0t.tile([P, 2048], dt.float16, tag="pB", name=f"p2_{i}")
            nc.vector.tensor_tensor(out=p1[:], in0=fr_ap, in1=fq_r[i][:], op=ALU.mult)
            nc.vector.tensor_tensor(out=p2[:], in0=fi_ap, in1=fq_i[i][:], op=ALU.mult)
            nc.vector.tensor_tensor(out=U0r[i][:], in0=p1[:], in1=p2[:], op=ALU.subtract)
            p3 = u0t.tile([P, 2048], dt.float16, tag="pA", name=f"p3_{i}")
            p4 = u0t.tile([P, 2048], dt.float16, tag="pB", name=f"p4_{i}")
            nc.vector.tensor_tensor(out=p3[:], in0=fr_ap, in1=fq_i[i][:], op=ALU.mult)
            nc.vector.tensor_tensor(out=p4[:], in0=fi_ap, in1=fq_r[i][:], op=ALU.mult)
            nc.vector.tensor_tensor(out=U0i[i][:], in0=p3[:], in1=p4[:], op=ALU.add)
            nc.vector.tensor_tensor(out=U0s[i][:], in0=U0r[i][:], in1=U0i[i][:], op=ALU.add)

        # es pair tiles alias the fq_r buffers (fq dead once U0 is produced)
        es_pair = [pp.tile([P, 2048], dt.float16, tag=f"fqr{i}", name=f"es{i}")
                   for i in range(4)]
        for kt in range(NB):
            sl = slice(P * kt, P * (kt + 1))
            nc.sync.dma_start(es_ap(kt, 0, 1024), es_d[sl, :])
        nc.sync.dma_start(f0[:], f0all_d)

        # ---- step 1 h0: mt 0-3, 4-mult, 8 banks, same-bank MM pairs ----
        psR = {mt: psp.tile([P, HN], dt.float32, tag=f"ps{mt}", name=f"ps1R{mt}")
               for mt in range(4)}
        psI = {mt: psp.tile([P, HN], dt.float32, tag=f"ps{mt + 4}", name=f"ps1I{mt}")
               for mt in range(4)}
        for ki, kt in enumerate(KT_ORDER):
            st = (ki == 0)
            sp = (ki == NB - 1)
            for mt in range(4):
                nc.tensor.matmul(psR[mt][:], lhsT=u0slice(U0r, kt, mt),
                                 rhs=er_ap(kt, 0, HN), start=st, stop=False)
                nc.tensor.matmul(psR[mt][:], lhsT=u0slice(U0i, kt, mt),
                                 rhs=nei_ap(kt), start=False, stop=sp)
                nc.tensor.matmul(psI[mt][:], lhsT=u0slice(U0r, kt, mt),
                                 rhs=ei_ap(kt, 0, HN), start=st, stop=False)
                nc.tensor.matmul(psI[mt][:], lhsT=u0slice(U0i, kt, mt),
                                 rhs=er_ap(kt, 0, HN), start=False, stop=sp)
        for mt in range(4):
            nc.scalar.activation(X1r[mt][:], psR[mt][:], AF.Identity)
            nc.scalar.activation(X1i[mt][:], psI[mt][:], AF.Identity)
            nc.vector.tensor_tensor(out=X1s[mt][:], in0=X1r[mt][:], in1=X1i[mt][:],
                                    op=ALU.add)

        # ---- step 1 h1: mt 4-7, Karatsuba, per-term kt sweeps ----
        ps_tag = 0

        def next_tags(n):
            nonlocal ps_tag
            tags = [(ps_tag + j) % NB for j in range(n)]
            ps_tag = (ps_tag + n) % NB
            return tags

        for rnd in range(2):
            mts = (4 + 2 * rnd, 5 + 2 * rnd)
            tags = next_tags(6)
            for j, mt in enumerate(mts):
                pa = psp.tile([P, HN], dt.float32, tag=f"ps{tags[3 * j]}", name=f"pA{mt}")
                pb = psp.tile([P, HN], dt.float32, tag=f"ps{tags[3 * j + 1]}", name=f"pB{mt}")
                pc = psp.tile([P, HN], dt.float32, tag=f"ps{tags[3 * j + 2]}", name=f"pC{mt}")
                for ki, kt in enumerate(KT_ORDER):
                    nc.tensor.matmul(pa[:], lhsT=u0slice(U0r, kt, mt),
                                     rhs=er_ap(kt, 0, HN), start=(ki == 0), stop=(ki == NB - 1))
                for ki, kt in enumerate(KT_ORDER):
                    nc.tensor.matmul(pb[:], lhsT=u0slice(U0i, kt, mt),
                                     rhs=ei_ap(kt, 0, HN), start=(ki == 0), stop=(ki == NB - 1))
                for ki, kt in enumerate(KT_ORDER):
                    nc.tensor.matmul(pc[:], lhsT=u0slice(U0s, kt, mt),
                                     rhs=es_ap(kt, 0, HN), start=(ki == 0), stop=(ki == NB - 1))
                bts = tp1.tile([P, HN], dt.float16, tag="bdr", name=f"b{mt}")
                nc.scalar.activation(bts[:], pb[:], AF.Identity)
                nc.vector.scalar_tensor_tensor(out=X1r[mt][:], in0=bts[:], scalar=-1.0,
                                               in1=pa[:], op0=ALU.mult, op1=ALU.add)
                nc.vector.scalar_tensor_tensor(out=X1s[mt][:], in0=bts[:], scalar=-2.0,
                                               in1=pc[:], op0=ALU.mult, op1=ALU.add)
                nc.vector.tensor_tensor(out=X1i[mt][:], in0=X1s[mt][:], in1=X1r[mt][:],
                                        op=ALU.subtract)

        # ---- step 2: Karatsuba per m2, per-term kt sweeps ----
        for m2 in range(NB):
            tags = next_tags(3)
            pa2 = psp.tile([P, HN], dt.float32, tag=f"ps{tags[0]}", name=f"pA2_{m2}")
            pb2 = psp.tile([P, HN], dt.float32, tag=f"ps{tags[1]}", name=f"pB2_{m2}")
            pc2 = psp.tile([P, HN], dt.float32, tag=f"ps{tags[2]}", name=f"pC2_{m2}")
            lo, hi = P * m2, P * (m2 + 1)
            for kt in range(NB):
                nc.tensor.matmul(pa2[:], lhsT=er_ap(kt, lo, hi), rhs=X1r[kt][:],
                                 start=(kt == 0), stop=(kt == NB - 1))
            for kt in range(NB):
                nc.tensor.matmul(pb2[:], lhsT=ei_ap(kt, lo, hi), rhs=X1i[kt][:],
                                 start=(kt == 0), stop=(kt == NB - 1))
            for kt in range(NB):
                nc.tensor.matmul(pc2[:], lhsT=es_ap(kt, lo, hi), rhs=X1s[kt][:],
                                 start=(kt == 0), stop=(kt == NB - 1))
            a2 = tp1.tile([P, HN], dt.float16, tag="a2", name=f"a2_{m2}")
            nc.scalar.activation(a2[:], pa2[:], AF.Identity)
            x2r = tp1.tile([P, HN], dt.float16, tag="x2r", name=f"x2r{m2}")
            x2i = tp1.tile([P, HN], dt.float16, tag="x2i", name=f"x2i{m2}")
            tq = tp1.tile([P, HN], dt.float16, tag="tq", name=f"tq{m2}")
            qlo = HN * (m2 % 4)
            f0r_ap = f0[:, qlo:qlo + HN]
            f0i_ap = f0[:, 2048 + qlo:2048 + qlo + HN]
            t1 = tp.tile([P, HN], dt.float16, tag="tA", name=f"t1_{m2}")
            t2 = tp.tile([P, HN], dt.float16, tag="tA", name=f"t2_{m2}")
            t3 = tp.tile([P, HN], dt.float16, tag="tB", name=f"t3_{m2}")
            t4 = tp.tile([P, HN], dt.float16, tag="tB", name=f"t4_{m2}")
            yb = tp.tile([P, 1024], dt.float16, tag="yb", name=f"yb{m2}")
            nc.vector.scalar_tensor_tensor(out=x2r[:], in0=a2[:], scalar=1.0,
                                           in1=pb2[:], op0=ALU.mult, op1=ALU.subtract)
            nc.vector.tensor_tensor(out=t1[:], in0=f0r_ap, in1=x2r[:], op=ALU.mult)
            nc.vector.tensor_tensor(out=t4[:], in0=f0i_ap, in1=x2r[:], op=ALU.mult)
            nc.vector.scalar_tensor_tensor(out=tq[:], in0=a2[:], scalar=-1.0,
                                           in1=pc2[:], op0=ALU.mult, op1=ALU.add)
            nc.vector.scalar_tensor_tensor(out=x2i[:], in0=tq[:], scalar=1.0,
                                           in1=pb2[:], op0=ALU.mult, op1=ALU.subtract)
            nc.vector.tensor_tensor(out=t2[:], in0=f0i_ap, in1=x2i[:], op=ALU.mult)
            nc.vector.tensor_tensor(out=t3[:], in0=f0r_ap, in1=x2i[:], op=ALU.mult)
            nc.vector.tensor_tensor(out=yb[:, 0:HN], in0=t1[:], in1=t2[:], op=ALU.subtract)
            nc.vector.tensor_tensor(out=yb[:, HN:1024], in0=t3[:], in1=t4[:], op=ALU.add)
            nc.sync.dma_start(y_d[P * m2:P * (m2 + 1), :], yb[:])

    nc.compile()
    return nc


def get_program():
    global _PROGRAM
    if _PROGRAM is None:
        _PROGRAM = build_program()
    return _PROGRAM


def kernel(field_real, field_imag, wavelengths):
    field_real = np.asarray(field_real)
    field_imag = np.asarray(field_imag)
    wavelengths = np.asarray(wavelengths)
    in_maps, meta = host_prepare(field_real, field_imag, wavelengths)
    from concourse.bass_utils import run_bass_kernel_spmd
    nc = get_program()
    res = run_bass_kernel_spmd(nc, in_maps, core_ids=list(range(8)))
    return assemble(res.results, meta)


if __name__ == "__main__":
    import jax
    import reference as ref
    cpu = jax.devices("cpu")[0]
    with jax.default_device(cpu):
        inputs = {k: np.asarray(v) for k, v in ref.setup_inputs().items()}
        expected = np.asarray(ref.reference(**{k: jax.device_put(v, cpu)
                                               for k, v in inputs.items()}))
    got = golden(np.asarray(inputs["field_real"]), np.asarray(inputs["field_imag"]),
                 np.asarray(inputs["wavelengths"]))
    err = np.abs(got - expected)
    print(f"golden absmax err {err.max():.4g} rel {err.max() / np.abs(expected).max():.4g}")
